# revision 6
# baseline (speedup 1.0000x reference)
"""GRU single-step kernel for Trainium2, data-parallel over 8 NeuronCores.

Computes h_next = GRUCell(x, h_prev) with PyTorch gate layout [r; z; n]:
    gi = x @ W_ih.T + b_ih ; gh = h @ W_hh.T + b_hh
    r = sigmoid(gi_r + gh_r); z = sigmoid(gi_z + gh_z)
    n = tanh(gi_n + r * gh_n); h' = (1-z)*n + z*h

Strategy: shard batch (16384 -> 8 x 2048). Weights replicated, pre-transposed
and bf16-cast on host so they stream as the matmul moving operand straight
from SBUF. Activations pre-transposed on host into the PE-stationary layout
(feature dim on partitions), so the device does zero transposes. PSUM holds
r/z/n_i/n_h pre-activations per 128-row x 512-col half-tile (4 banks, double
buffered = all 8 banks). Epilogue on DVE/ACT; tanh(x) = 2*sigmoid(2x)-1 so the
ACT engine never reloads its function table.
"""

import os
import sys

import numpy as np

if "/opt/trn_rl_repo" not in sys.path:
    sys.path.insert(0, "/opt/trn_rl_repo")

H = 1024           # hidden == input size
B = 16384
NCORES = 8
BLOC = B // NCORES  # 2048 rows per core
P = 128
NTILES = BLOC // P  # 16 row tiles per core
KC = H // P         # 8 contraction chunks
NG = 512            # matmul free dim / PSUM bank width (fp32)

_cache = {}


def _build_program():
    from concourse import bacc, bass, mybir, tile

    f32 = mybir.dt.float32
    bf16 = mybir.dt.bfloat16
    Alu = mybir.AluOpType
    ActFn = mybir.ActivationFunctionType

    nc = bacc.Bacc("TRN2", target_bir_lowering=False, debug=False)

    # DRAM parameters (per-core shapes)
    xT = nc.declare_dram_parameter("xT", [NTILES, P, H], bf16, isOutput=False)
    hT = nc.declare_dram_parameter("hT", [NTILES, P, H], bf16, isOutput=False)
    hN = nc.declare_dram_parameter("hN", [NTILES, P, H], f32, isOutput=False)
    # wT[m*KC+kc] : [P, 3H] slice of W_m.T   (m=0 -> ih, m=1 -> hh)
    wT = nc.declare_dram_parameter("wT", [2 * KC, P, 3 * H], bf16, isOutput=False)
    # bias_b : broadcast biases [P, 4H] = [r_comb | z_comb | n_i | n_h]
    bias_b = nc.declare_dram_parameter("bias_b", [P, 4 * H], f32, isOutput=False)
    out = nc.declare_dram_parameter("h_next", [NTILES, P, H], f32, isOutput=True)

    with tile.TileContext(nc) as tc:
        with (
            tc.tile_pool(name="wpool", bufs=1) as wpool,
            tc.tile_pool(name="stream", bufs=3) as stream,
            tc.tile_pool(name="temps", bufs=2) as temps,
            tc.tile_pool(name="psum", bufs=2, space="PSUM") as psum,
        ):
            # Startup DMAs are chunked and emitted in the order the PE will
            # consume them, so the first matmul can start within a few us and
            # no single fat transfer serializes one DMA queue.
            w_tiles = [wpool.tile([P, 3 * H], bf16, tag=f"w{i}", name=f"w{i}") for i in range(2 * KC)]
            bias_t = wpool.tile([P, 4 * H], f32, tag="bias")

            # tile 0 stationary operands first (32KB per chunk)
            xt0 = stream.tile([P, KC, P], bf16, tag="xt")
            ht0 = stream.tile([P, KC, P], bf16, tag="ht")
            for kc in range(KC):
                nc.sync.dma_start(out=xt0[:, kc, :], in_=xT[0, :, kc * P:(kc + 1) * P])
            for kc in range(KC):
                nc.sync.dma_start(out=ht0[:, kc, :], in_=hT[0, :, kc * P:(kc + 1) * P])
            # weight chunks for half A (cols j*512 with j in 0,2,4), x-side
            # weights before h-side, then bias A, hn0, then the B halves.
            for m in range(2):
                for kc in range(KC):
                    for j in (0, 2, 4):
                        nc.sync.dma_start(
                            out=w_tiles[m * KC + kc][:, j * NG:(j + 1) * NG],
                            in_=wT[m * KC + kc, :, j * NG:(j + 1) * NG])
            for j in (0, 2, 4, 6):
                nc.sync.dma_start(out=bias_t[:, j * NG:(j + 1) * NG],
                                  in_=bias_b[:, j * NG:(j + 1) * NG])
            hn0 = stream.tile([P, H], f32, tag="hn")
            for j in range(2):
                nc.sync.dma_start(out=hn0[:, j * NG:(j + 1) * NG],
                                  in_=hN[0, :, j * NG:(j + 1) * NG])
            for m in range(2):
                for kc in range(KC):
                    for j in (1, 3, 5):
                        nc.sync.dma_start(
                            out=w_tiles[m * KC + kc][:, j * NG:(j + 1) * NG],
                            in_=wT[m * KC + kc, :, j * NG:(j + 1) * NG])
            for j in (1, 3, 5, 7):
                nc.sync.dma_start(out=bias_t[:, j * NG:(j + 1) * NG],
                                  in_=bias_b[:, j * NG:(j + 1) * NG])

            for i in range(NTILES):
                if i == 0:
                    xt, ht, hn = xt0, ht0, hn0
                else:
                    xt = stream.tile([P, KC, P], bf16, tag="xt")
                    nc.sync.dma_start(out=xt[:], in_=xT[i])
                    ht = stream.tile([P, KC, P], bf16, tag="ht")
                    nc.sync.dma_start(out=ht[:], in_=hT[i])
                    hn = stream.tile([P, H], f32, tag="hn")
                    for j in range(2):
                        nc.sync.dma_start(out=hn[:, j * NG:(j + 1) * NG],
                                          in_=hN[i, :, j * NG:(j + 1) * NG])
                ot = stream.tile([P, H], f32, tag="ot")

                for half in range(2):
                    g0 = half * NG
                    R = psum.tile([P, NG], f32, tag="R")
                    Z = psum.tile([P, NG], f32, tag="Z")
                    NI = psum.tile([P, NG], f32, tag="NI")
                    NH_ = psum.tile([P, NG], f32, tag="NH")

                    # x-side: gi chunks (r, z, n_i)
                    for kc in range(KC):
                        st = xt[:, kc, :]
                        w = w_tiles[kc]
                        nc.tensor.matmul(R[:], st, w[:, g0:g0 + NG],
                                         start=(kc == 0), stop=False)
                        nc.tensor.matmul(Z[:], st, w[:, H + g0:H + g0 + NG],
                                         start=(kc == 0), stop=False)
                        nc.tensor.matmul(NI[:], st, w[:, 2 * H + g0:2 * H + g0 + NG],
                                         start=(kc == 0), stop=(kc == KC - 1))
                    # h-side: gh chunks (r, z accumulate; n_h separate)
                    for kc in range(KC):
                        st = ht[:, kc, :]
                        w = w_tiles[KC + kc]
                        nc.tensor.matmul(R[:], st, w[:, g0:g0 + NG],
                                         start=False, stop=(kc == KC - 1))
                        nc.tensor.matmul(Z[:], st, w[:, H + g0:H + g0 + NG],
                                         start=False, stop=(kc == KC - 1))
                        nc.tensor.matmul(NH_[:], st, w[:, 2 * H + g0:2 * H + g0 + NG],
                                         start=(kc == 0), stop=(kc == KC - 1))

                    # epilogue for this [128, 512] half
                    rpre = temps.tile([P, NG], f32, tag="rpre")
                    nc.vector.tensor_tensor(rpre[:], R[:], bias_t[:, g0:g0 + NG], Alu.add)
                    r = temps.tile([P, NG], f32, tag="r")
                    nc.scalar.activation(r[:], rpre[:], ActFn.Sigmoid)

                    zpre = temps.tile([P, NG], f32, tag="zpre")
                    nc.vector.tensor_tensor(zpre[:], Z[:], bias_t[:, H + g0:H + g0 + NG], Alu.add)
                    z = temps.tile([P, NG], f32, tag="z")
                    nc.scalar.activation(z[:], zpre[:], ActFn.Sigmoid)

                    u = temps.tile([P, NG], f32, tag="u")
                    nc.vector.tensor_tensor(u[:], NH_[:], bias_t[:, 3 * H + g0:3 * H + g0 + NG], Alu.add)
                    t = temps.tile([P, NG], f32, tag="t")
                    nc.vector.tensor_tensor(t[:], r[:], u[:], Alu.mult)
                    v = temps.tile([P, NG], f32, tag="v")
                    nc.vector.tensor_tensor(v[:], NI[:], bias_t[:, 2 * H + g0:2 * H + g0 + NG], Alu.add)
                    npre = temps.tile([P, NG], f32, tag="npre")
                    nc.vector.tensor_tensor(npre[:], v[:], t[:], Alu.add)

                    # n = tanh(npre) = 2*sigmoid(2*npre) - 1 (single ACT table)
                    s = temps.tile([P, NG], f32, tag="s")
                    nc.scalar.activation(s[:], npre[:], ActFn.Sigmoid, scale=2.0)
                    n = temps.tile([P, NG], f32, tag="n")
                    nc.vector.tensor_scalar(n[:], s[:], 2.0, -1.0, Alu.mult, Alu.add)

                    # h' = n + z*(h - n)
                    hm1 = temps.tile([P, NG], f32, tag="hm1")
                    nc.vector.tensor_tensor(hm1[:], hn[:, g0:g0 + NG], n[:], Alu.subtract)
                    hm2 = temps.tile([P, NG], f32, tag="hm2")
                    nc.vector.tensor_tensor(hm2[:], z[:], hm1[:], Alu.mult)
                    nc.vector.tensor_tensor(ot[:, g0:g0 + NG], n[:], hm2[:], Alu.add)

                nc.sync.dma_start(out=out[i], in_=ot[:])

    nc.compile()
    return nc


def _prep_inputs(x, h_prev, weight_ih, weight_hh, bias_ih, bias_hh):
    import ml_dtypes

    bf16 = ml_dtypes.bfloat16

    # activations -> [core, tile, p, kc, b] with value a[core*2048 + tile*128 + b, kc*128 + p]
    def to_stationary(a):
        v = a.reshape(NCORES, NTILES, P, KC, P).transpose(0, 1, 4, 3, 2)
        return np.ascontiguousarray(v).astype(bf16).reshape(NCORES, NTILES, P, H)

    xT = to_stationary(x)
    hT = to_stationary(h_prev)
    hN = np.ascontiguousarray(h_prev.reshape(NCORES, NTILES, P, H)).astype(np.float32)

    # weights -> W.T chunked: [m*KC+kc, p, g] = W_m[g, kc*128+p]
    def wt_chunks(w):
        return np.ascontiguousarray(w.T.reshape(KC, P, 3 * H)).astype(bf16)

    wT = np.concatenate([wt_chunks(weight_ih), wt_chunks(weight_hh)], axis=0)

    b_r = bias_ih[:H] + bias_hh[:H]
    b_z = bias_ih[H:2 * H] + bias_hh[H:2 * H]
    b_ni = bias_ih[2 * H:]
    b_nh = bias_hh[2 * H:]
    bias_vec = np.concatenate([b_r, b_z, b_ni, b_nh]).astype(np.float32)
    bias_b = np.ascontiguousarray(np.broadcast_to(bias_vec, (P, 4 * H)))

    in_maps = []
    for c in range(NCORES):
        in_maps.append({
            "xT": xT[c], "hT": hT[c], "hN": hN[c],
            "wT": wT, "bias_b": bias_b,
        })
    return in_maps


def kernel(x, h_prev, weight_ih, weight_hh, bias_ih, bias_hh):
    from concourse.bass_utils import run_bass_kernel_spmd

    x = np.asarray(x, dtype=np.float32)
    h_prev = np.asarray(h_prev, dtype=np.float32)
    weight_ih = np.asarray(weight_ih, dtype=np.float32)
    weight_hh = np.asarray(weight_hh, dtype=np.float32)
    bias_ih = np.asarray(bias_ih, dtype=np.float32)
    bias_hh = np.asarray(bias_hh, dtype=np.float32)

    if "nc" not in _cache:
        _cache["nc"] = _build_program()
    nc = _cache["nc"]

    in_maps = _prep_inputs(x, h_prev, weight_ih, weight_hh, bias_ih, bias_hh)
    trace = os.environ.get("GRU_TRACE", "0") == "1"
    res = run_bass_kernel_spmd(nc, in_maps, list(range(NCORES)), trace=trace)
    kernel._last_exec_ns = res.exec_time_ns

    outs = [np.asarray(res.results[c]["h_next"]).reshape(BLOC, H) for c in range(NCORES)]
    return np.concatenate(outs, axis=0).astype(np.float32)


kernel._last_exec_ns = None


# revision 7
# speedup vs baseline: 1.0363x; 1.0363x over previous
"""GRU single-step kernel for Trainium2, data-parallel over 8 NeuronCores.

Computes h_next = GRUCell(x, h_prev) with PyTorch gate layout [r; z; n]:
    gi = x @ W_ih.T + b_ih ; gh = h @ W_hh.T + b_hh
    r = sigmoid(gi_r + gh_r); z = sigmoid(gi_z + gh_z)
    n = tanh(gi_n + r * gh_n); h' = (1-z)*n + z*h

Strategy: shard batch (16384 -> 8 x 2048). Weights replicated, pre-transposed
and bf16-cast on host so they stream as the matmul moving operand straight
from SBUF. Activations pre-transposed on host into the PE-stationary layout
(feature dim on partitions), so the device does zero transposes. PSUM holds
r/z/n_i/n_h pre-activations per 128-row x 512-col half-tile (4 banks, double
buffered = all 8 banks). Epilogue on DVE/ACT; tanh(x) = 2*sigmoid(2x)-1 so the
ACT engine never reloads its function table.
"""

import os
import sys

import numpy as np

if "/opt/trn_rl_repo" not in sys.path:
    sys.path.insert(0, "/opt/trn_rl_repo")

H = 1024           # hidden == input size
B = 16384
NCORES = 8
BLOC = B // NCORES  # 2048 rows per core
P = 128
NTILES = BLOC // P  # 16 row tiles per core
KC = H // P         # 8 contraction chunks
NG = 512            # matmul free dim / PSUM bank width (fp32)

_cache = {}


def _build_program():
    from concourse import bacc, bass, mybir, tile

    f32 = mybir.dt.float32
    bf16 = mybir.dt.bfloat16
    Alu = mybir.AluOpType
    ActFn = mybir.ActivationFunctionType

    nc = bacc.Bacc("TRN2", target_bir_lowering=False, debug=False)

    # DRAM parameters (per-core shapes)
    xT = nc.declare_dram_parameter("xT", [NTILES, P, H], bf16, isOutput=False)
    hT = nc.declare_dram_parameter("hT", [NTILES, P, H], bf16, isOutput=False)
    hN = nc.declare_dram_parameter("hN", [NTILES, P, H], f32, isOutput=False)
    # wT[m*KC+kc] : [P, 3H] slice of W_m.T   (m=0 -> ih, m=1 -> hh)
    wT = nc.declare_dram_parameter("wT", [2 * KC, P, 3 * H], bf16, isOutput=False)
    # bias_b : broadcast biases [P, 4H] = [r_comb | z_comb | n_i | n_h]
    bias_b = nc.declare_dram_parameter("bias_b", [P, 4 * H], f32, isOutput=False)
    out = nc.declare_dram_parameter("h_next", [NTILES, P, H], f32, isOutput=True)

    with tile.TileContext(nc) as tc:
        with (
            tc.tile_pool(name="wpool", bufs=1) as wpool,
            tc.tile_pool(name="stream", bufs=3) as stream,
            tc.tile_pool(name="temps", bufs=2) as temps,
            tc.tile_pool(name="psum", bufs=2, space="PSUM") as psum,
        ):
            # Startup DMAs are chunked and emitted in the order the PE will
            # consume them, so the first matmul can start within a few us and
            # no single fat transfer serializes one DMA queue.
            w_tiles = [wpool.tile([P, 3 * H], bf16, tag=f"w{i}", name=f"w{i}") for i in range(2 * KC)]
            bias_t = wpool.tile([P, 4 * H], f32, tag="bias")

            # tile 0 stationary operands first (32KB per chunk)
            xt0 = stream.tile([P, KC, P], bf16, tag="xt")
            ht0 = stream.tile([P, KC, P], bf16, tag="ht")
            for kc in range(KC):
                nc.gpsimd.dma_start(out=xt0[:, kc, :], in_=xT[0, :, kc * P:(kc + 1) * P])
            for kc in range(KC):
                nc.gpsimd.dma_start(out=ht0[:, kc, :], in_=hT[0, :, kc * P:(kc + 1) * P])
            # weight chunks for half A (cols j*512 with j in 0,2,4), x-side
            # weights before h-side, then bias A, hn0, then the B halves.
            for m in range(2):
                for kc in range(KC):
                    for j in (0, 2, 4):
                        nc.sync.dma_start(
                            out=w_tiles[m * KC + kc][:, j * NG:(j + 1) * NG],
                            in_=wT[m * KC + kc, :, j * NG:(j + 1) * NG])
            for j in (0, 2, 4, 6):
                nc.sync.dma_start(out=bias_t[:, j * NG:(j + 1) * NG],
                                  in_=bias_b[:, j * NG:(j + 1) * NG])
            hn0 = stream.tile([P, H], f32, tag="hn")
            for j in range(2):
                nc.gpsimd.dma_start(out=hn0[:, j * NG:(j + 1) * NG],
                                  in_=hN[0, :, j * NG:(j + 1) * NG])
            for m in range(2):
                for kc in range(KC):
                    for j in (1, 3, 5):
                        nc.sync.dma_start(
                            out=w_tiles[m * KC + kc][:, j * NG:(j + 1) * NG],
                            in_=wT[m * KC + kc, :, j * NG:(j + 1) * NG])
            for j in (1, 3, 5, 7):
                nc.sync.dma_start(out=bias_t[:, j * NG:(j + 1) * NG],
                                  in_=bias_b[:, j * NG:(j + 1) * NG])

            for i in range(NTILES):
                if i == 0:
                    xt, ht, hn = xt0, ht0, hn0
                else:
                    xt = stream.tile([P, KC, P], bf16, tag="xt")
                    nc.gpsimd.dma_start(out=xt[:], in_=xT[i])
                    ht = stream.tile([P, KC, P], bf16, tag="ht")
                    nc.gpsimd.dma_start(out=ht[:], in_=hT[i])
                    hn = stream.tile([P, H], f32, tag="hn")
                    nc.gpsimd.dma_start(out=hn[:], in_=hN[i])
                ot = stream.tile([P, H], f32, tag="ot")

                for half in range(2):
                    g0 = half * NG
                    R = psum.tile([P, NG], f32, tag="R")
                    Z = psum.tile([P, NG], f32, tag="Z")
                    NI = psum.tile([P, NG], f32, tag="NI")
                    NH_ = psum.tile([P, NG], f32, tag="NH")

                    # x-side: gi chunks (r, z, n_i)
                    for kc in range(KC):
                        st = xt[:, kc, :]
                        w = w_tiles[kc]
                        nc.tensor.matmul(R[:], st, w[:, g0:g0 + NG],
                                         start=(kc == 0), stop=False)
                        nc.tensor.matmul(Z[:], st, w[:, H + g0:H + g0 + NG],
                                         start=(kc == 0), stop=False)
                        nc.tensor.matmul(NI[:], st, w[:, 2 * H + g0:2 * H + g0 + NG],
                                         start=(kc == 0), stop=(kc == KC - 1))
                    # h-side: gh chunks (r, z accumulate; n_h separate)
                    for kc in range(KC):
                        st = ht[:, kc, :]
                        w = w_tiles[KC + kc]
                        nc.tensor.matmul(R[:], st, w[:, g0:g0 + NG],
                                         start=False, stop=(kc == KC - 1))
                        nc.tensor.matmul(Z[:], st, w[:, H + g0:H + g0 + NG],
                                         start=False, stop=(kc == KC - 1))
                        nc.tensor.matmul(NH_[:], st, w[:, 2 * H + g0:2 * H + g0 + NG],
                                         start=(kc == 0), stop=(kc == KC - 1))

                    # epilogue for this [128, 512] half
                    rpre = temps.tile([P, NG], f32, tag="rpre")
                    nc.vector.tensor_tensor(rpre[:], R[:], bias_t[:, g0:g0 + NG], Alu.add)
                    r = temps.tile([P, NG], f32, tag="r")
                    nc.scalar.activation(r[:], rpre[:], ActFn.Sigmoid)

                    zpre = temps.tile([P, NG], f32, tag="zpre")
                    nc.vector.tensor_tensor(zpre[:], Z[:], bias_t[:, H + g0:H + g0 + NG], Alu.add)
                    z = temps.tile([P, NG], f32, tag="z")
                    nc.scalar.activation(z[:], zpre[:], ActFn.Sigmoid)

                    u = temps.tile([P, NG], f32, tag="u")
                    nc.vector.tensor_tensor(u[:], NH_[:], bias_t[:, 3 * H + g0:3 * H + g0 + NG], Alu.add)
                    t = temps.tile([P, NG], f32, tag="t")
                    nc.vector.tensor_tensor(t[:], r[:], u[:], Alu.mult)
                    v = temps.tile([P, NG], f32, tag="v")
                    nc.vector.tensor_tensor(v[:], NI[:], bias_t[:, 2 * H + g0:2 * H + g0 + NG], Alu.add)
                    npre = temps.tile([P, NG], f32, tag="npre")
                    nc.vector.tensor_tensor(npre[:], v[:], t[:], Alu.add)

                    # n = tanh(npre) = 2*sigmoid(2*npre) - 1 (single ACT table)
                    s = temps.tile([P, NG], f32, tag="s")
                    nc.scalar.activation(s[:], npre[:], ActFn.Sigmoid, scale=2.0)
                    n = temps.tile([P, NG], f32, tag="n")
                    nc.vector.tensor_scalar(n[:], s[:], 2.0, -1.0, Alu.mult, Alu.add)

                    # h' = n + z*(h - n)
                    hm1 = temps.tile([P, NG], f32, tag="hm1")
                    nc.vector.tensor_tensor(hm1[:], hn[:, g0:g0 + NG], n[:], Alu.subtract)
                    hm2 = temps.tile([P, NG], f32, tag="hm2")
                    nc.vector.tensor_tensor(hm2[:], z[:], hm1[:], Alu.mult)
                    nc.vector.tensor_tensor(ot[:, g0:g0 + NG], n[:], hm2[:], Alu.add)

                nc.gpsimd.dma_start(out=out[i], in_=ot[:])

    nc.compile()
    return nc


def _prep_inputs(x, h_prev, weight_ih, weight_hh, bias_ih, bias_hh):
    import ml_dtypes

    bf16 = ml_dtypes.bfloat16

    # activations -> [core, tile, p, kc, b] with value a[core*2048 + tile*128 + b, kc*128 + p]
    def to_stationary(a):
        v = a.reshape(NCORES, NTILES, P, KC, P).transpose(0, 1, 4, 3, 2)
        return np.ascontiguousarray(v).astype(bf16).reshape(NCORES, NTILES, P, H)

    xT = to_stationary(x)
    hT = to_stationary(h_prev)
    hN = np.ascontiguousarray(h_prev.reshape(NCORES, NTILES, P, H)).astype(np.float32)

    # weights -> W.T chunked: [m*KC+kc, p, g] = W_m[g, kc*128+p]
    def wt_chunks(w):
        return np.ascontiguousarray(w.T.reshape(KC, P, 3 * H)).astype(bf16)

    wT = np.concatenate([wt_chunks(weight_ih), wt_chunks(weight_hh)], axis=0)

    b_r = bias_ih[:H] + bias_hh[:H]
    b_z = bias_ih[H:2 * H] + bias_hh[H:2 * H]
    b_ni = bias_ih[2 * H:]
    b_nh = bias_hh[2 * H:]
    bias_vec = np.concatenate([b_r, b_z, b_ni, b_nh]).astype(np.float32)
    bias_b = np.ascontiguousarray(np.broadcast_to(bias_vec, (P, 4 * H)))

    in_maps = []
    for c in range(NCORES):
        in_maps.append({
            "xT": xT[c], "hT": hT[c], "hN": hN[c],
            "wT": wT, "bias_b": bias_b,
        })
    return in_maps


def kernel(x, h_prev, weight_ih, weight_hh, bias_ih, bias_hh):
    from concourse.bass_utils import run_bass_kernel_spmd

    x = np.asarray(x, dtype=np.float32)
    h_prev = np.asarray(h_prev, dtype=np.float32)
    weight_ih = np.asarray(weight_ih, dtype=np.float32)
    weight_hh = np.asarray(weight_hh, dtype=np.float32)
    bias_ih = np.asarray(bias_ih, dtype=np.float32)
    bias_hh = np.asarray(bias_hh, dtype=np.float32)

    if "nc" not in _cache:
        _cache["nc"] = _build_program()
    nc = _cache["nc"]

    in_maps = _prep_inputs(x, h_prev, weight_ih, weight_hh, bias_ih, bias_hh)
    trace = os.environ.get("GRU_TRACE", "0") == "1"
    res = run_bass_kernel_spmd(nc, in_maps, list(range(NCORES)), trace=trace)
    kernel._last_exec_ns = res.exec_time_ns

    outs = [np.asarray(res.results[c]["h_next"]).reshape(BLOC, H) for c in range(NCORES)]
    return np.concatenate(outs, axis=0).astype(np.float32)


kernel._last_exec_ns = None


# revision 8
# speedup vs baseline: 1.0486x; 1.0119x over previous
"""GRU single-step kernel for Trainium2, data-parallel over 8 NeuronCores.

Computes h_next = GRUCell(x, h_prev) with PyTorch gate layout [r; z; n]:
    gi = x @ W_ih.T + b_ih ; gh = h @ W_hh.T + b_hh
    r = sigmoid(gi_r + gh_r); z = sigmoid(gi_z + gh_z)
    n = tanh(gi_n + r * gh_n); h' = (1-z)*n + z*h

Strategy: shard batch (16384 -> 8 x 2048). Weights replicated, pre-transposed
and bf16-cast on host so they stream as the matmul moving operand straight
from SBUF. Activations pre-transposed on host into the PE-stationary layout
(feature dim on partitions), so the device does zero transposes. PSUM holds
r/z/n_i/n_h pre-activations per 128-row x 512-col half-tile (4 banks, double
buffered = all 8 banks). Epilogue on DVE/ACT; tanh(x) = 2*sigmoid(2x)-1 so the
ACT engine never reloads its function table.
"""

import os
import sys

import numpy as np

if "/opt/trn_rl_repo" not in sys.path:
    sys.path.insert(0, "/opt/trn_rl_repo")

H = 1024           # hidden == input size
B = 16384
NCORES = 8
BLOC = B // NCORES  # 2048 rows per core
P = 128
NTILES = BLOC // P  # 16 row tiles per core
KC = H // P         # 8 contraction chunks
NG = 512            # matmul free dim / PSUM bank width (fp32)

_cache = {}


def _build_program():
    from concourse import bacc, bass, mybir, tile

    f32 = mybir.dt.float32
    bf16 = mybir.dt.bfloat16
    Alu = mybir.AluOpType
    ActFn = mybir.ActivationFunctionType

    nc = bacc.Bacc("TRN2", target_bir_lowering=False, debug=False)

    # DRAM parameters (per-core shapes)
    xT = nc.declare_dram_parameter("xT", [NTILES, P, H], bf16, isOutput=False)
    hT = nc.declare_dram_parameter("hT", [NTILES, P, H], bf16, isOutput=False)
    hN = nc.declare_dram_parameter("hN", [NTILES, P, H], f32, isOutput=False)
    # wT[m*KC+kc] : [P, 3H] slice of W_m.T   (m=0 -> ih, m=1 -> hh)
    wT = nc.declare_dram_parameter("wT", [2 * KC, P, 3 * H], bf16, isOutput=False)
    # bias_b : broadcast biases [P, 4H] = [r_comb | z_comb | n_i | n_h]
    bias_b = nc.declare_dram_parameter("bias_b", [P, 4 * H], f32, isOutput=False)
    out = nc.declare_dram_parameter("h_next", [NTILES, P, H], f32, isOutput=True)

    with tile.TileContext(nc) as tc:
        with (
            tc.tile_pool(name="wpool", bufs=1) as wpool,
            tc.tile_pool(name="stream", bufs=3) as stream,
            tc.tile_pool(name="temps", bufs=2) as temps,
            tc.tile_pool(name="psum", bufs=2, space="PSUM") as psum,
        ):
            # Startup DMAs are chunked and emitted in the order the PE will
            # consume them, so the first matmul can start within a few us and
            # no single fat transfer serializes one DMA queue.
            w_tiles = [wpool.tile([P, 3 * H], bf16, tag=f"w{i}", name=f"w{i}") for i in range(2 * KC)]
            bias_t = wpool.tile([P, 4 * H], f32, tag="bias")

            # tile 0 stationary operands first (32KB per chunk)
            xt0 = stream.tile([P, KC, P], bf16, tag="xt")
            ht0 = stream.tile([P, KC, P], bf16, tag="ht")
            for kc in range(KC):
                nc.gpsimd.dma_start(out=xt0[:, kc, :], in_=xT[0, :, kc * P:(kc + 1) * P])
            for kc in range(KC):
                nc.gpsimd.dma_start(out=ht0[:, kc, :], in_=hT[0, :, kc * P:(kc + 1) * P])
            # weight chunks for half A (cols j*512 with j in 0,2,4), x-side
            # weights before h-side, then bias A, hn0, then the B halves.
            for m in range(2):
                for kc in range(KC):
                    for j in (0, 2, 4):
                        nc.sync.dma_start(
                            out=w_tiles[m * KC + kc][:, j * NG:(j + 1) * NG],
                            in_=wT[m * KC + kc, :, j * NG:(j + 1) * NG])
            for j in (0, 2, 4, 6):
                nc.sync.dma_start(out=bias_t[:, j * NG:(j + 1) * NG],
                                  in_=bias_b[:, j * NG:(j + 1) * NG])
            hn0 = stream.tile([P, H], f32, tag="hn")
            for j in range(2):
                nc.gpsimd.dma_start(out=hn0[:, j * NG:(j + 1) * NG],
                                  in_=hN[0, :, j * NG:(j + 1) * NG])
            for m in range(2):
                for kc in range(KC):
                    for j in (1, 3, 5):
                        nc.sync.dma_start(
                            out=w_tiles[m * KC + kc][:, j * NG:(j + 1) * NG],
                            in_=wT[m * KC + kc, :, j * NG:(j + 1) * NG])
            for j in (1, 3, 5, 7):
                nc.sync.dma_start(out=bias_t[:, j * NG:(j + 1) * NG],
                                  in_=bias_b[:, j * NG:(j + 1) * NG])

            for i in range(NTILES):
                if i == 0:
                    xt, ht, hn = xt0, ht0, hn0
                else:
                    xt = stream.tile([P, KC, P], bf16, tag="xt")
                    nc.gpsimd.dma_start(out=xt[:], in_=xT[i])
                    ht = stream.tile([P, KC, P], bf16, tag="ht")
                    nc.gpsimd.dma_start(out=ht[:], in_=hT[i])
                    hn = stream.tile([P, H], f32, tag="hn")
                    nc.gpsimd.dma_start(out=hn[:], in_=hN[i])
                ot = stream.tile([P, H], f32, tag="ot")

                for half in range(2):
                    g0 = half * NG
                    R = psum.tile([P, NG], f32, tag="R")
                    Z = psum.tile([P, NG], f32, tag="Z")
                    NI = psum.tile([P, NG], f32, tag="NI")
                    NH_ = psum.tile([P, NG], f32, tag="NH")

                    # x-side: gi chunks (r, z, n_i)
                    for kc in range(KC):
                        st = xt[:, kc, :]
                        w = w_tiles[kc]
                        nc.tensor.matmul(R[:], st, w[:, g0:g0 + NG],
                                         start=(kc == 0), stop=False)
                        nc.tensor.matmul(Z[:], st, w[:, H + g0:H + g0 + NG],
                                         start=(kc == 0), stop=False)
                        nc.tensor.matmul(NI[:], st, w[:, 2 * H + g0:2 * H + g0 + NG],
                                         start=(kc == 0), stop=(kc == KC - 1))
                    # h-side: gh chunks (r, z accumulate; n_h separate)
                    for kc in range(KC):
                        st = ht[:, kc, :]
                        w = w_tiles[KC + kc]
                        nc.tensor.matmul(R[:], st, w[:, g0:g0 + NG],
                                         start=False, stop=(kc == KC - 1))
                        nc.tensor.matmul(Z[:], st, w[:, H + g0:H + g0 + NG],
                                         start=False, stop=(kc == KC - 1))
                        nc.tensor.matmul(NH_[:], st, w[:, 2 * H + g0:2 * H + g0 + NG],
                                         start=(kc == 0), stop=(kc == KC - 1))

                    # epilogue for this [128, 512] half
                    rpre = temps.tile([P, NG], f32, tag="rpre")
                    nc.vector.tensor_tensor(rpre[:], R[:], bias_t[:, g0:g0 + NG], Alu.add)
                    r = temps.tile([P, NG], f32, tag="r")
                    nc.scalar.activation(r[:], rpre[:], ActFn.Sigmoid)

                    zpre = temps.tile([P, NG], f32, tag="zpre")
                    nc.vector.tensor_tensor(zpre[:], Z[:], bias_t[:, H + g0:H + g0 + NG], Alu.add)
                    z = temps.tile([P, NG], f32, tag="z")
                    nc.scalar.activation(z[:], zpre[:], ActFn.Sigmoid)

                    u = temps.tile([P, NG], f32, tag="u")
                    nc.vector.tensor_tensor(u[:], NH_[:], bias_t[:, 3 * H + g0:3 * H + g0 + NG], Alu.add)
                    t = temps.tile([P, NG], f32, tag="t")
                    nc.vector.tensor_tensor(t[:], r[:], u[:], Alu.mult)
                    v = temps.tile([P, NG], f32, tag="v")
                    nc.vector.tensor_tensor(v[:], NI[:], bias_t[:, 2 * H + g0:2 * H + g0 + NG], Alu.add)
                    npre = temps.tile([P, NG], f32, tag="npre")
                    nc.vector.tensor_tensor(npre[:], v[:], t[:], Alu.add)

                    # n = tanh(npre) = 2*sigmoid(2*npre) - 1 (single ACT table)
                    s = temps.tile([P, NG], f32, tag="s")
                    nc.scalar.activation(s[:], npre[:], ActFn.Sigmoid, scale=2.0)
                    n = temps.tile([P, NG], f32, tag="n")
                    nc.vector.tensor_scalar(n[:], s[:], 2.0, -1.0, Alu.mult, Alu.add)

                    # h' = n + z*(h - n)
                    hm1 = temps.tile([P, NG], f32, tag="hm1")
                    nc.vector.tensor_tensor(hm1[:], hn[:, g0:g0 + NG], n[:], Alu.subtract)
                    hm2 = temps.tile([P, NG], f32, tag="hm2")
                    nc.vector.tensor_tensor(hm2[:], z[:], hm1[:], Alu.mult)
                    nc.vector.tensor_tensor(ot[:, g0:g0 + NG], n[:], hm2[:], Alu.add)

                nc.sync.dma_start(out=out[i], in_=ot[:])

    nc.compile()
    return nc


def _prep_inputs(x, h_prev, weight_ih, weight_hh, bias_ih, bias_hh):
    import ml_dtypes

    bf16 = ml_dtypes.bfloat16

    # activations -> [core, tile, p, kc, b] with value a[core*2048 + tile*128 + b, kc*128 + p]
    def to_stationary(a):
        v = a.reshape(NCORES, NTILES, P, KC, P).transpose(0, 1, 4, 3, 2)
        return np.ascontiguousarray(v).astype(bf16).reshape(NCORES, NTILES, P, H)

    xT = to_stationary(x)
    hT = to_stationary(h_prev)
    hN = np.ascontiguousarray(h_prev.reshape(NCORES, NTILES, P, H)).astype(np.float32)

    # weights -> W.T chunked: [m*KC+kc, p, g] = W_m[g, kc*128+p]
    def wt_chunks(w):
        return np.ascontiguousarray(w.T.reshape(KC, P, 3 * H)).astype(bf16)

    wT = np.concatenate([wt_chunks(weight_ih), wt_chunks(weight_hh)], axis=0)

    b_r = bias_ih[:H] + bias_hh[:H]
    b_z = bias_ih[H:2 * H] + bias_hh[H:2 * H]
    b_ni = bias_ih[2 * H:]
    b_nh = bias_hh[2 * H:]
    bias_vec = np.concatenate([b_r, b_z, b_ni, b_nh]).astype(np.float32)
    bias_b = np.ascontiguousarray(np.broadcast_to(bias_vec, (P, 4 * H)))

    in_maps = []
    for c in range(NCORES):
        in_maps.append({
            "xT": xT[c], "hT": hT[c], "hN": hN[c],
            "wT": wT, "bias_b": bias_b,
        })
    return in_maps


def kernel(x, h_prev, weight_ih, weight_hh, bias_ih, bias_hh):
    from concourse.bass_utils import run_bass_kernel_spmd

    x = np.asarray(x, dtype=np.float32)
    h_prev = np.asarray(h_prev, dtype=np.float32)
    weight_ih = np.asarray(weight_ih, dtype=np.float32)
    weight_hh = np.asarray(weight_hh, dtype=np.float32)
    bias_ih = np.asarray(bias_ih, dtype=np.float32)
    bias_hh = np.asarray(bias_hh, dtype=np.float32)

    if "nc" not in _cache:
        _cache["nc"] = _build_program()
    nc = _cache["nc"]

    in_maps = _prep_inputs(x, h_prev, weight_ih, weight_hh, bias_ih, bias_hh)
    trace = os.environ.get("GRU_TRACE", "0") == "1"
    res = run_bass_kernel_spmd(nc, in_maps, list(range(NCORES)), trace=trace)
    kernel._last_exec_ns = res.exec_time_ns

    outs = [np.asarray(res.results[c]["h_next"]).reshape(BLOC, H) for c in range(NCORES)]
    return np.concatenate(outs, axis=0).astype(np.float32)


kernel._last_exec_ns = None


# revision 10
# speedup vs baseline: 1.0531x; 1.0043x over previous
"""GRU single-step kernel for Trainium2, data-parallel over 8 NeuronCores.

Computes h_next = GRUCell(x, h_prev) with PyTorch gate layout [r; z; n]:
    gi = x @ W_ih.T + b_ih ; gh = h @ W_hh.T + b_hh
    r = sigmoid(gi_r + gh_r); z = sigmoid(gi_z + gh_z)
    n = tanh(gi_n + r * gh_n); h' = (1-z)*n + z*h

Strategy: shard batch (16384 -> 8 x 2048). Weights replicated, pre-transposed
and bf16-cast on host so they stream as the matmul moving operand straight
from SBUF. Activations pre-transposed on host into the PE-stationary layout
(feature dim on partitions), so the device does zero transposes. PSUM holds
r/z/n_i/n_h pre-activations per 128-row x 512-col half-tile (4 banks, double
buffered = all 8 banks). Epilogue on DVE/ACT; tanh(x) = 2*sigmoid(2x)-1 so the
ACT engine never reloads its function table.
"""

import os
import sys

import numpy as np

if "/opt/trn_rl_repo" not in sys.path:
    sys.path.insert(0, "/opt/trn_rl_repo")

H = 1024           # hidden == input size
B = 16384
NCORES = 8
BLOC = B // NCORES  # 2048 rows per core
P = 128
NTILES = BLOC // P  # 16 row tiles per core
KC = H // P         # 8 contraction chunks
NG = 512            # matmul free dim / PSUM bank width (fp32)

_cache = {}


def _build_program():
    from concourse import bacc, bass, mybir, tile

    f32 = mybir.dt.float32
    bf16 = mybir.dt.bfloat16
    Alu = mybir.AluOpType
    ActFn = mybir.ActivationFunctionType

    nc = bacc.Bacc("TRN2", target_bir_lowering=False, debug=False)

    # DRAM parameters (per-core shapes)
    xT = nc.declare_dram_parameter("xT", [NTILES, P, H], bf16, isOutput=False)
    hT = nc.declare_dram_parameter("hT", [NTILES, P, H], bf16, isOutput=False)
    hN = nc.declare_dram_parameter("hN", [NTILES, P, H], f32, isOutput=False)
    # wT[m*KC+kc] : [P, 3H] slice of W_m.T   (m=0 -> ih, m=1 -> hh)
    wT = nc.declare_dram_parameter("wT", [2 * KC, P, 3 * H], bf16, isOutput=False)
    # bias_b : broadcast biases [P, 4H] = [r_comb | z_comb | n_i | n_h]
    bias_b = nc.declare_dram_parameter("bias_b", [P, 4 * H], f32, isOutput=False)
    out = nc.declare_dram_parameter("h_next", [NTILES, P, H], f32, isOutput=True)

    with tile.TileContext(nc) as tc:
        with (
            tc.tile_pool(name="wpool", bufs=1) as wpool,
            tc.tile_pool(name="stream", bufs=3) as stream,
            tc.tile_pool(name="temps", bufs=2) as temps,
            tc.tile_pool(name="psum", bufs=2, space="PSUM") as psum,
        ):
            # Startup DMAs are chunked and emitted in the order the PE will
            # consume them, so the first matmul can start within a few us and
            # no single fat transfer serializes one DMA queue.
            w_tiles = [wpool.tile([P, 3 * H], bf16, tag=f"w{i}", name=f"w{i}") for i in range(2 * KC)]
            bias_t = wpool.tile([P, 4 * H], f32, tag="bias")

            # tile 0 stationary operands first (32KB per chunk)
            xt0 = stream.tile([P, KC, P], bf16, tag="xt")
            ht0 = stream.tile([P, KC, P], bf16, tag="ht")
            for kc in range(KC):
                nc.gpsimd.dma_start(out=xt0[:, kc, :], in_=xT[0, :, kc * P:(kc + 1) * P])
            for kc in range(KC):
                nc.gpsimd.dma_start(out=ht0[:, kc, :], in_=hT[0, :, kc * P:(kc + 1) * P])
            # weight chunks for half A (cols j*512 with j in 0,2,4), x-side
            # weights before h-side, then bias A, hn0, then the B halves.
            for m in range(2):
                for kc in range(KC):
                    for j in (0, 2, 4):
                        nc.sync.dma_start(
                            out=w_tiles[m * KC + kc][:, j * NG:(j + 1) * NG],
                            in_=wT[m * KC + kc, :, j * NG:(j + 1) * NG])
            for j in (0, 2, 4, 6):
                nc.sync.dma_start(out=bias_t[:, j * NG:(j + 1) * NG],
                                  in_=bias_b[:, j * NG:(j + 1) * NG])
            hn0 = stream.tile([P, H], f32, tag="hn")
            for j in range(2):
                nc.gpsimd.dma_start(out=hn0[:, j * NG:(j + 1) * NG],
                                  in_=hN[0, :, j * NG:(j + 1) * NG])
            for m in range(2):
                for kc in range(KC):
                    for j in (1, 3, 5):
                        nc.sync.dma_start(
                            out=w_tiles[m * KC + kc][:, j * NG:(j + 1) * NG],
                            in_=wT[m * KC + kc, :, j * NG:(j + 1) * NG])
            for j in (1, 3, 5, 7):
                nc.sync.dma_start(out=bias_t[:, j * NG:(j + 1) * NG],
                                  in_=bias_b[:, j * NG:(j + 1) * NG])

            for i in range(NTILES):
                if i == 0:
                    xt, ht, hn = xt0, ht0, hn0
                else:
                    xt = stream.tile([P, KC, P], bf16, tag="xt")
                    nc.gpsimd.dma_start(out=xt[:], in_=xT[i])
                    ht = stream.tile([P, KC, P], bf16, tag="ht")
                    nc.gpsimd.dma_start(out=ht[:], in_=hT[i])
                    hn = stream.tile([P, H], f32, tag="hn")
                    nc.gpsimd.dma_start(out=hn[:], in_=hN[i])
                ot = stream.tile([P, H], f32, tag="ot")

                for half in range(2):
                    g0 = half * NG
                    R = psum.tile([P, NG], f32, tag="R")
                    Z = psum.tile([P, NG], f32, tag="Z")
                    NI = psum.tile([P, NG], f32, tag="NI")
                    NH_ = psum.tile([P, NG], f32, tag="NH")

                    # x-side: gi chunks (r, z, n_i)
                    for kc in range(KC):
                        st = xt[:, kc, :]
                        w = w_tiles[kc]
                        nc.tensor.matmul(R[:], st, w[:, g0:g0 + NG],
                                         start=(kc == 0), stop=False)
                        nc.tensor.matmul(Z[:], st, w[:, H + g0:H + g0 + NG],
                                         start=(kc == 0), stop=False)
                        nc.tensor.matmul(NI[:], st, w[:, 2 * H + g0:2 * H + g0 + NG],
                                         start=(kc == 0), stop=(kc == KC - 1))
                    # h-side: gh chunks (r, z accumulate; n_h separate)
                    for kc in range(KC):
                        st = ht[:, kc, :]
                        w = w_tiles[KC + kc]
                        nc.tensor.matmul(R[:], st, w[:, g0:g0 + NG],
                                         start=False, stop=(kc == KC - 1))
                        nc.tensor.matmul(Z[:], st, w[:, H + g0:H + g0 + NG],
                                         start=False, stop=(kc == KC - 1))
                        nc.tensor.matmul(NH_[:], st, w[:, 2 * H + g0:2 * H + g0 + NG],
                                         start=(kc == 0), stop=(kc == KC - 1))

                    # epilogue for this [128, 512] half
                    rpre = temps.tile([P, NG], f32, tag="rpre")
                    nc.vector.tensor_tensor(rpre[:], R[:], bias_t[:, g0:g0 + NG], Alu.add)
                    r = temps.tile([P, NG], f32, tag="r")
                    nc.scalar.activation(r[:], rpre[:], ActFn.Sigmoid)

                    zpre = temps.tile([P, NG], f32, tag="zpre")
                    nc.vector.tensor_tensor(zpre[:], Z[:], bias_t[:, H + g0:H + g0 + NG], Alu.add)
                    z = temps.tile([P, NG], f32, tag="z")
                    nc.scalar.activation(z[:], zpre[:], ActFn.Sigmoid)

                    u = temps.tile([P, NG], f32, tag="u")
                    nc.vector.tensor_tensor(u[:], NH_[:], bias_t[:, 3 * H + g0:3 * H + g0 + NG], Alu.add)
                    t = temps.tile([P, NG], f32, tag="t")
                    nc.vector.tensor_tensor(t[:], r[:], u[:], Alu.mult)
                    v = temps.tile([P, NG], f32, tag="v")
                    nc.vector.tensor_tensor(v[:], NI[:], bias_t[:, 2 * H + g0:2 * H + g0 + NG], Alu.add)
                    npre = temps.tile([P, NG], f32, tag="npre")
                    nc.vector.tensor_tensor(npre[:], v[:], t[:], Alu.add)

                    # n = tanh(npre) = 2*sigmoid(2*npre) - 1 (single ACT table)
                    s = temps.tile([P, NG], f32, tag="s")
                    nc.scalar.activation(s[:], npre[:], ActFn.Sigmoid, scale=2.0)
                    n = temps.tile([P, NG], f32, tag="n")
                    nc.vector.tensor_scalar(n[:], s[:], 2.0, -1.0, Alu.mult, Alu.add)

                    # h' = n + z*(h - n)
                    hm1 = temps.tile([P, NG], f32, tag="hm1")
                    nc.vector.tensor_tensor(hm1[:], hn[:, g0:g0 + NG], n[:], Alu.subtract)
                    hm2 = temps.tile([P, NG], f32, tag="hm2")
                    nc.vector.tensor_tensor(hm2[:], z[:], hm1[:], Alu.mult)
                    nc.vector.tensor_tensor(ot[:, g0:g0 + NG], n[:], hm2[:], Alu.add)
                    nc.sync.dma_start(out=out[i, :, g0:g0 + NG], in_=ot[:, g0:g0 + NG])

    nc.compile()
    return nc


def _prep_inputs(x, h_prev, weight_ih, weight_hh, bias_ih, bias_hh):
    import ml_dtypes

    bf16 = ml_dtypes.bfloat16

    # activations -> [core, tile, p, kc, b] with value a[core*2048 + tile*128 + b, kc*128 + p]
    def to_stationary(a):
        v = a.reshape(NCORES, NTILES, P, KC, P).transpose(0, 1, 4, 3, 2)
        return np.ascontiguousarray(v).astype(bf16).reshape(NCORES, NTILES, P, H)

    xT = to_stationary(x)
    hT = to_stationary(h_prev)
    hN = np.ascontiguousarray(h_prev.reshape(NCORES, NTILES, P, H)).astype(np.float32)

    # weights -> W.T chunked: [m*KC+kc, p, g] = W_m[g, kc*128+p]
    def wt_chunks(w):
        return np.ascontiguousarray(w.T.reshape(KC, P, 3 * H)).astype(bf16)

    wT = np.concatenate([wt_chunks(weight_ih), wt_chunks(weight_hh)], axis=0)

    b_r = bias_ih[:H] + bias_hh[:H]
    b_z = bias_ih[H:2 * H] + bias_hh[H:2 * H]
    b_ni = bias_ih[2 * H:]
    b_nh = bias_hh[2 * H:]
    bias_vec = np.concatenate([b_r, b_z, b_ni, b_nh]).astype(np.float32)
    bias_b = np.ascontiguousarray(np.broadcast_to(bias_vec, (P, 4 * H)))

    in_maps = []
    for c in range(NCORES):
        in_maps.append({
            "xT": xT[c], "hT": hT[c], "hN": hN[c],
            "wT": wT, "bias_b": bias_b,
        })
    return in_maps


def kernel(x, h_prev, weight_ih, weight_hh, bias_ih, bias_hh):
    from concourse.bass_utils import run_bass_kernel_spmd

    x = np.asarray(x, dtype=np.float32)
    h_prev = np.asarray(h_prev, dtype=np.float32)
    weight_ih = np.asarray(weight_ih, dtype=np.float32)
    weight_hh = np.asarray(weight_hh, dtype=np.float32)
    bias_ih = np.asarray(bias_ih, dtype=np.float32)
    bias_hh = np.asarray(bias_hh, dtype=np.float32)

    if "nc" not in _cache:
        _cache["nc"] = _build_program()
    nc = _cache["nc"]

    in_maps = _prep_inputs(x, h_prev, weight_ih, weight_hh, bias_ih, bias_hh)
    trace = os.environ.get("GRU_TRACE", "0") == "1"
    res = run_bass_kernel_spmd(nc, in_maps, list(range(NCORES)), trace=trace)
    kernel._last_exec_ns = res.exec_time_ns

    outs = [np.asarray(res.results[c]["h_next"]).reshape(BLOC, H) for c in range(NCORES)]
    return np.concatenate(outs, axis=0).astype(np.float32)


kernel._last_exec_ns = None
